# revision 7
# baseline (speedup 1.0000x reference)
"""Trainium2 Bass kernel for nn_AdExternal_N3Tree (gnn_message_passing).

Key insight: the reference's sequential 32768-step scan only affects the
output through `acc` (a 32-vector). Each parent's 8-child sibling group is an
independent serial chain that is LINEAR in that parent's original 8x32 block,
and group depth is constant within each of 6 contiguous parent-index classes.
So  acc = sum_d M_d @ s_d + gamma_tot,  where s_d is the sum of flattened
parent blocks over class d (a pure 4MB reduction) and M_d (32x256) / gamma
are tiny matrices computed on the host from conv_w/conv_b/depth_weight alone.

The leaf cells that feed the MLPs (flat cells 32767..262143) are never
written by the scan, so  out[leaf] = MLP(data_flat[leaf] + acc)  and cells
0..32766 are zero.

Device work per core (SPMD on 8 cores):
  - stage 1: partial class sums over a 512-node scan shard (f32 matmuls
    against a 0/1 class-indicator), AllGather + on-PE sum -> s (6x256)
  - tiny matmuls: s -> transpose -> acc -> folded layer-1 bias (128,)
  - MLP over a 29184-cell slice in bf16: x@W1cat (row-tiled) -> GELU+bias on
    ScalarE (the throughput bound, f32 PSUM in) -> @W2cat (col-tiled) ->
    +b2 evict on DVE
  - output written as 12 (c,o)-planes; host interleaves/assembles.
"""
import sys

for _p in ("/opt/trn_rl_repo", "/opt/trn_rl_repo/concourse"):
    if _p not in sys.path:
        sys.path.insert(0, _p)

import numpy as np

N_NODES = 32768
D = 32
N_GROUPS = 4096
N_CELLS = N_NODES * 8           # 262144
LEAF0 = N_NODES - 1             # 32767 first leaf cell
CORES = 8
CELLS_MAIN = 28672              # cells each core contributes (core 7: +1)
CELLS_CORE = 29184              # cells each core processes = 19 * 1536
CH = 1536                       # cells per chunk (3 row-tiled sub-chunks of 512)
NCH = 19
SUB = 512
NSUB = 3
SCAN_TILES = 32                 # replicated scan: 32 node-tiles of 128
SCAN_FREE = SCAN_TILES * 256    # 8192
XT_FREE = NCH * SUB             # 9728

# (p_lo, p_hi_inclusive, conv_depth, n_children, extra_j0_step)
CLASSES = [
    (0, 0, 1, 8, True),
    (1, 8, 2, 8, False),
    (9, 72, 3, 8, False),
    (73, 584, 4, 8, False),
    (585, 4094, 5, 8, False),
    (4095, 4095, 5, 7, False),
]


# ---------------------------------------------------------------- host math
def _chain(conv_w_d, conv_b_d, dw_d, n_children):
    W = conv_w_d.astype(np.float64)
    b = conv_b_d.astype(np.float64)
    Wk = [W[:, :, k] for k in range(8)]
    A, beta = {}, {}
    if n_children == 7:
        A7 = np.zeros((8, D, D))
        A7[7] = np.eye(D)
        A[7] = A7
        beta[7] = np.zeros(D)
        cs = range(6, -1, -1)
    else:
        cs = range(7, -1, -1)
    for c in cs:
        Ac = np.zeros((8, D, D))
        bc = b.copy()
        for k in range(0, c + 1):
            Ac[k] += Wk[k]
        for m in range(c + 1, 8):
            for k in range(8):
                Ac[k] += Wk[m] @ A[m][k]
            bc += Wk[m] @ beta[m]
        A[c] = Ac
        beta[c] = bc
    Msum = np.zeros((8, D, D))
    gamma = np.zeros(D)
    for c in (range(8) if n_children == 8 else range(7)):
        Msum += dw_d * A[c]
        gamma += dw_d * beta[c]
    return A, beta, Msum, gamma


def _build_class_mats(conv_w, conv_b, depth_weight):
    out = []
    for (p_lo, p_hi, dep, nch, extra) in CLASSES:
        A, beta, Msum, gamma = _chain(
            conv_w[dep], conv_b[dep], float(depth_weight[dep]), nch
        )
        if extra:
            W0 = conv_w[0].astype(np.float64)
            b0 = conv_b[0].astype(np.float64)
            W0k = [W0[:, :, k] for k in range(8)]
            Ae = np.zeros((8, D, D))
            be = b0.copy()
            for m in range(8):
                for k in range(8):
                    Ae[k] += W0k[m] @ A[m][k]
                be += W0k[m] @ beta[m]
            Msum = Msum + float(depth_weight[0]) * Ae
            gamma = gamma + float(depth_weight[0]) * be
        M = np.concatenate([Msum[k] for k in range(8)], axis=1)  # (D, 8D)
        out.append((p_lo, p_hi, M, gamma))
    return out


# ---------------------------------------------------------------- device graph
_GRAPH = None


def _build_graph():
    import concourse.bacc as bacc
    import concourse.mybir as mybir
    from concourse import tile

    F32 = mybir.dt.float32
    BF16 = mybir.dt.bfloat16
    nc = bacc.Bacc("TRN2", target_bir_lowering=False, debug=False, num_devices=CORES)

    xT_d = nc.declare_dram_parameter("xT", [96, XT_FREE], BF16, isOutput=False)
    scan_d = nc.declare_dram_parameter("scanX", [128, SCAN_FREE], BF16, isOutput=False)
    ind_d = nc.declare_dram_parameter("ind", [128, 192], BF16, isOutput=False)
    m2_d = nc.declare_dram_parameter("m2", [128, 384], F32, isOutput=False)
    wb_d = nc.declare_dram_parameter("wb", [33, 128], F32, isOutput=False)
    w1_d = nc.declare_dram_parameter("w1", [96, 128], BF16, isOutput=False)
    w2_d = nc.declare_dram_parameter("w2", [128, 4], BF16, isOutput=False)
    b2_d = nc.declare_dram_parameter("b2col", [128, 1], F32, isOutput=False)
    id_d = nc.declare_dram_parameter("ident", [6, 6], F32, isOutput=False)
    one_d = nc.declare_dram_parameter("one", [1, 1], F32, isOutput=False)
    out_d = nc.declare_dram_parameter("out", [12, XT_FREE], F32, isOutput=True)

    Gelu = mybir.ActivationFunctionType.Gelu

    with tile.TileContext(nc) as tc:
        with (
            tc.tile_pool(name="const", bufs=1) as cpool,
            tc.tile_pool(name="data", bufs=1) as dpool,
            tc.tile_pool(name="gp", bufs=3) as gpool,
            tc.tile_pool(name="dram", bufs=1, space="DRAM") as drpool,
        ):
            # --- ACT warm-up: force the gelu table load at t~0 ---
            warm_sb = cpool.tile([1, 8], F32)
            nc.gpsimd.memset(warm_sb[:], 0.0)
            nc.scalar.activation(warm_sb[:], warm_sb[:], Gelu)

            ind_sb = cpool.tile([128, 192], BF16)
            m2_sb = cpool.tile([128, 384], F32)
            wb_sb = cpool.tile([33, 128], F32)
            w1_sb = cpool.tile([96, 128], BF16)
            w2_sb = cpool.tile([128, 4], BF16)
            b2_sb = cpool.tile([128, 1], F32)
            id_sb = cpool.tile([6, 6], F32)
            acc1 = cpool.tile([33, 1], F32)
            bias_sb = cpool.tile([128, 1], F32)
            s_sb = cpool.tile([6, 256], F32)
            sT_sb = cpool.tile([128, 12], F32)

            # ALL input DMAs ride the scalar HWDGE queue (it starts ~6us
            # earlier than the sync queue); strict FIFO: consts, scan, xT.
            nc.scalar.dma_start(ind_sb[:], ind_d.ap())
            nc.scalar.dma_start(m2_sb[:], m2_d.ap())
            nc.scalar.dma_start(wb_sb[:], wb_d.ap())
            nc.scalar.dma_start(w1_sb[:], w1_d.ap())
            nc.scalar.dma_start(w2_sb[:], w2_d.ap())
            nc.scalar.dma_start(b2_sb[:], b2_d.ap())
            nc.scalar.dma_start(id_sb[:], id_d.ap())
            nc.scalar.dma_start(acc1[32:33, :], one_d.ap())
            scan_sb = dpool.tile([128, SCAN_FREE], BF16)
            for q in range(4):
                nc.scalar.dma_start(
                    scan_sb[:, q * 2048:(q + 1) * 2048],
                    scan_d.ap()[:, q * 2048:(q + 1) * 2048],
                )
            xT_sb = dpool.tile([96, XT_FREE], BF16)
            for q in range(4):
                nc.scalar.dma_start(
                    xT_sb[:, q * 2432:(q + 1) * 2432],
                    xT_d.ap()[:, q * 2432:(q + 1) * 2432],
                )
            stage_sb = dpool.tile([128, XT_FREE], F32)

            with tc.tile_pool(name="psA", bufs=1, space="PSUM") as psA:
                # stage 1: class sums over the full (replicated) scan region
                ps_s = psA.tile([6, 256], F32)
                for T in range(SCAN_TILES):
                    nc.tensor.matmul(
                        ps_s[:],
                        ind_sb[:, 6 * T:6 * T + 6],
                        scan_sb[:, 256 * T:256 * (T + 1)],
                        start=(T == 0),
                        stop=(T == SCAN_TILES - 1),
                    )
                nc.vector.tensor_copy(s_sb[:], ps_s[:])

                # transpose s (6,256) -> sT (128,12) via two identity matmuls
                ps_T = psA.tile([128, 12], F32)
                for jhi in range(2):
                    nc.tensor.matmul(
                        ps_T[:, 6 * jhi:6 * jhi + 6],
                        s_sb[:, 128 * jhi:128 * (jhi + 1)],
                        id_sb[:],
                        start=True,
                        stop=True,
                    )
                nc.vector.tensor_copy(sT_sb[:], ps_T[:])

                # acc = sum_k M2_k @ sT[:, k]
                ps_acc = psA.tile([32, 1], F32)
                for k in range(12):
                    nc.tensor.matmul(
                        ps_acc[:],
                        m2_sb[:, 32 * k:32 * (k + 1)],
                        sT_sb[:, k:k + 1],
                        start=(k == 0),
                        stop=(k == 11),
                    )
                nc.vector.tensor_copy(acc1[0:32, :], ps_acc[:])

                # bias1_eff = W1cat.T @ acc + (b1cat + gamma_tot @ W1cat)
                ps_b = psA.tile([128, 1], F32)
                nc.tensor.matmul(ps_b[:], wb_sb[:], acc1[:], start=True, stop=True)
                nc.vector.tensor_copy(bias_sb[:], ps_b[:])

            with (
                tc.tile_pool(name="psZ", bufs=2, space="PSUM") as zp,
                tc.tile_pool(name="psO", bufs=2, space="PSUM") as op,
            ):
                for t in range(NCH):
                    z = zp.tile([128, CH], F32)
                    for a in range(NSUB):
                        nc.tensor.matmul(
                            z[:, SUB * a:SUB * (a + 1)],
                            w1_sb[32 * a:32 * (a + 1), :],
                            xT_sb[32 * a:32 * (a + 1), SUB * t:SUB * (t + 1)],
                            start=True,
                            stop=True,
                            tile_position=(32 * a, 0),
                        )
                    g = gpool.tile([128, CH], BF16)
                    nc.scalar.activation(g[:], z[:], Gelu, bias=bias_sb[:])
                    o_ps = op.tile([128, SUB], F32)
                    for c in range(NSUB):
                        nc.tensor.matmul(
                            o_ps[32 * c:32 * c + 4, :],
                            w2_sb[:],
                            g[:, SUB * c:SUB * (c + 1)],
                            start=True,
                            stop=True,
                            tile_position=(0, 32 * c),
                        )
                    nc.vector.tensor_scalar_add(
                        stage_sb[:, SUB * t:SUB * (t + 1)], o_ps[:], b2_sb[:]
                    )
                    # batched output DMA on the idle gpsimd queue
                    if t in (4, 9, 14, 17, NCH - 1):
                        lo = {4: 0, 9: 2560, 14: 5120, 17: 7680, NCH - 1: 9216}[t]
                        hi = SUB * (t + 1)
                        for c in range(NSUB):
                            nc.gpsimd.dma_start(
                                out_d.ap()[4 * c:4 * c + 4, lo:hi],
                                stage_sb[32 * c:32 * c + 4, lo:hi],
                            )

    nc.compile()
    return nc


def _get_graph():
    global _GRAPH
    if _GRAPH is None:
        _GRAPH = _build_graph()
    return _GRAPH


# ---------------------------------------------------------------- kernel
def kernel(**inputs):
    import ml_dtypes
    from concourse import bass_utils

    data = np.asarray(inputs["data"], np.float32)
    conv_w = np.asarray(inputs["conv_w"], np.float32)
    conv_b = np.asarray(inputs["conv_b"], np.float32)
    dw = np.asarray(inputs["depth_weight"], np.float32)
    f_w1 = np.asarray(inputs["f_w1"], np.float32)
    f_b1 = np.asarray(inputs["f_b1"], np.float32)
    f_w2 = np.asarray(inputs["f_w2"], np.float32)
    f_b2 = np.asarray(inputs["f_b2"], np.float32)
    s_w1 = np.asarray(inputs["s_w1"], np.float32)
    s_b1 = np.asarray(inputs["s_b1"], np.float32)
    s_w2 = np.asarray(inputs["s_w2"], np.float32)
    s_b2 = np.asarray(inputs["s_b2"], np.float32)

    # --- weight-derived host constants (no data-sized work here) ---
    mats = _build_class_mats(conv_w, conv_b, dw)

    W1cat = np.concatenate([f_w1, s_w1], axis=1)          # (32, 128)
    b1cat = np.concatenate([f_b1, s_b1])                  # (128,)
    gamma_tot = np.zeros(D)
    for (p_lo, p_hi, M, gamma) in mats:
        gamma_tot += (p_hi - p_lo + 1) * gamma
    bconst = b1cat.astype(np.float64) + gamma_tot @ W1cat.astype(np.float64)
    WB = np.concatenate(
        [W1cat, bconst[None, :].astype(np.float32)], axis=0
    ).astype(np.float32)                                   # (33, 128)

    W2cat = np.zeros((128, 4), np.float32)
    W2cat[0:64, 0:3] = f_w2
    W2cat[64:128, 3:4] = s_w2
    b2cat = np.concatenate([f_b2, s_b2]).astype(np.float32)
    b2col = np.zeros((128, 1), np.float32)
    for c in range(NSUB):
        b2col[32 * c:32 * c + 4, 0] = b2cat

    # M2 (128, 384): col block k=6*jhi+d : M2[jlo, 32k+o] = M_d[o, 128*jhi+jlo]
    m2 = np.zeros((128, 384), np.float32)
    for dcls, (p_lo, p_hi, M, gamma) in enumerate(mats):
        Mf = M.astype(np.float32)
        for jhi in range(2):
            k = 6 * jhi + dcls
            m2[:, 32 * k:32 * (k + 1)] = Mf[:, 128 * jhi:128 * (jhi + 1)].T

    # --- shards ---
    data_flat = data.reshape(N_CELLS, D)
    w1_np = np.ascontiguousarray(
        np.tile(W1cat, (3, 1)).astype(ml_dtypes.bfloat16)
    )

    # replicated scan region (all 4096 parent nodes), bf16
    scan = np.ascontiguousarray(
        data_flat[0:N_GROUPS * 8].reshape(SCAN_TILES, 128, 256).transpose(1, 0, 2)
        .reshape(128, SCAN_FREE).astype(ml_dtypes.bfloat16)
    )
    ind = np.zeros((128, 192), np.float32)
    for dcls, (p_lo, p_hi, M, gamma) in enumerate(mats):
        for node in range(p_lo, p_hi + 1):
            T, p = divmod(node, 128)
            ind[p, 6 * T + dcls] = 1.0
    ind = ind.astype(ml_dtypes.bfloat16)

    in_maps = []
    for i in range(CORES):
        base = LEAF0 + CELLS_MAIN * i
        end = min(base + CELLS_CORE, N_CELLS)
        x_lin = np.zeros((CELLS_CORE, D), np.float32)
        x_lin[0:end - base] = data_flat[base:end]
        xT = np.ascontiguousarray(
            x_lin.reshape(NCH, NSUB, SUB, D).transpose(1, 3, 0, 2)
            .reshape(96, XT_FREE).astype(ml_dtypes.bfloat16)
        )
        in_maps.append({
            "xT": xT,
            "scanX": scan,
            "ind": ind,
            "m2": m2,
            "wb": WB,
            "w1": w1_np,
            "w2": W2cat.astype(ml_dtypes.bfloat16),
            "b2col": b2col,
            "ident": np.eye(6, dtype=np.float32),
            "one": np.ones((1, 1), np.float32),
        })

    nc = _get_graph()
    res = bass_utils.run_bass_kernel_spmd(nc, in_maps, core_ids=list(range(CORES)))

    out_flat = np.zeros((N_CELLS, 4), np.float32)
    for i in range(CORES):
        base = LEAF0 + CELLS_MAIN * i
        k = CELLS_MAIN if i < CORES - 1 else CELLS_MAIN + 1
        # planes (12, 9728): row 4c+o holds cells 1536t+512c+cc at free 512t+cc
        planes = res.results[i]["out"].reshape(NSUB, 4, NCH, SUB)  # (c,o,t,cc)
        cells = planes.transpose(2, 0, 3, 1).reshape(CELLS_CORE, 4)  # (t,c,cc),o
        out_flat[base:base + k] = cells[:k]
    return out_flat.reshape(N_NODES, 2, 2, 2, 4)


# revision 9
# speedup vs baseline: 1.0363x; 1.0363x over previous
"""Trainium2 Bass kernel for nn_AdExternal_N3Tree (gnn_message_passing).

Key insight: the reference's sequential 32768-step scan only affects the
output through `acc` (a 32-vector). Each parent's 8-child sibling group is an
independent serial chain that is LINEAR in that parent's original 8x32 block,
and group depth is constant within each of 6 contiguous parent-index classes.
So  acc = sum_d M_d @ s_d + gamma_tot,  where s_d is the sum of flattened
parent blocks over class d (a pure 4MB reduction) and M_d (32x256) / gamma
are tiny matrices computed on the host from conv_w/conv_b/depth_weight alone.

The leaf cells that feed the MLPs (flat cells 32767..262143) are never
written by the scan, so  out[leaf] = MLP(data_flat[leaf] + acc)  and cells
0..32766 are zero.

Device work per core (SPMD on 8 cores):
  - stage 1: partial class sums over a 512-node scan shard (f32 matmuls
    against a 0/1 class-indicator), AllGather + on-PE sum -> s (6x256)
  - tiny matmuls: s -> transpose -> acc -> folded layer-1 bias (128,)
  - MLP over a 29184-cell slice in bf16: x@W1cat (row-tiled) -> GELU+bias on
    ScalarE (the throughput bound, f32 PSUM in) -> @W2cat (col-tiled) ->
    +b2 evict on DVE
  - output written as 12 (c,o)-planes; host interleaves/assembles.
"""
import sys

for _p in ("/opt/trn_rl_repo", "/opt/trn_rl_repo/concourse"):
    if _p not in sys.path:
        sys.path.insert(0, _p)

import numpy as np

N_NODES = 32768
D = 32
N_GROUPS = 4096
N_CELLS = N_NODES * 8           # 262144
LEAF0 = N_NODES - 1             # 32767 first leaf cell
CORES = 8
CELLS_MAIN = 28672              # cells each core contributes (core 7: +1)
CELLS_CORE = 29184              # cells each core processes = 19 * 1536
CH = 1536                       # cells per chunk (3 row-tiled sub-chunks of 512)
NCH = 19
SUB = 512
NSUB = 3
SCAN_TILES = 32                 # replicated scan: 32 node-tiles of 128
SCAN_FREE = SCAN_TILES * 256    # 8192
XT_FREE = NCH * SUB             # 9728

# (p_lo, p_hi_inclusive, conv_depth, n_children, extra_j0_step)
CLASSES = [
    (0, 0, 1, 8, True),
    (1, 8, 2, 8, False),
    (9, 72, 3, 8, False),
    (73, 584, 4, 8, False),
    (585, 4094, 5, 8, False),
    (4095, 4095, 5, 7, False),
]


# ---------------------------------------------------------------- host math
def _chain(conv_w_d, conv_b_d, dw_d, n_children):
    W = conv_w_d.astype(np.float64)
    b = conv_b_d.astype(np.float64)
    Wk = [W[:, :, k] for k in range(8)]
    A, beta = {}, {}
    if n_children == 7:
        A7 = np.zeros((8, D, D))
        A7[7] = np.eye(D)
        A[7] = A7
        beta[7] = np.zeros(D)
        cs = range(6, -1, -1)
    else:
        cs = range(7, -1, -1)
    for c in cs:
        Ac = np.zeros((8, D, D))
        bc = b.copy()
        for k in range(0, c + 1):
            Ac[k] += Wk[k]
        for m in range(c + 1, 8):
            for k in range(8):
                Ac[k] += Wk[m] @ A[m][k]
            bc += Wk[m] @ beta[m]
        A[c] = Ac
        beta[c] = bc
    Msum = np.zeros((8, D, D))
    gamma = np.zeros(D)
    for c in (range(8) if n_children == 8 else range(7)):
        Msum += dw_d * A[c]
        gamma += dw_d * beta[c]
    return A, beta, Msum, gamma


def _build_class_mats(conv_w, conv_b, depth_weight):
    out = []
    for (p_lo, p_hi, dep, nch, extra) in CLASSES:
        A, beta, Msum, gamma = _chain(
            conv_w[dep], conv_b[dep], float(depth_weight[dep]), nch
        )
        if extra:
            W0 = conv_w[0].astype(np.float64)
            b0 = conv_b[0].astype(np.float64)
            W0k = [W0[:, :, k] for k in range(8)]
            Ae = np.zeros((8, D, D))
            be = b0.copy()
            for m in range(8):
                for k in range(8):
                    Ae[k] += W0k[m] @ A[m][k]
                be += W0k[m] @ beta[m]
            Msum = Msum + float(depth_weight[0]) * Ae
            gamma = gamma + float(depth_weight[0]) * be
        M = np.concatenate([Msum[k] for k in range(8)], axis=1)  # (D, 8D)
        out.append((p_lo, p_hi, M, gamma))
    return out


# ---------------------------------------------------------------- device graph
_GRAPH = None


def _build_graph():
    import concourse.bacc as bacc
    import concourse.mybir as mybir
    from concourse import tile
    from concourse.tile_rust import add_dep_helper

    F32 = mybir.dt.float32
    BF16 = mybir.dt.bfloat16
    nc = bacc.Bacc("TRN2", target_bir_lowering=False, debug=False, num_devices=CORES)

    xT_d = nc.declare_dram_parameter("xT", [96, XT_FREE], BF16, isOutput=False)
    scan_d = nc.declare_dram_parameter("scanX", [128, SCAN_FREE], BF16, isOutput=False)
    ind_d = nc.declare_dram_parameter("ind", [128, 192], BF16, isOutput=False)
    m2_d = nc.declare_dram_parameter("m2", [128, 384], F32, isOutput=False)
    wb_d = nc.declare_dram_parameter("wb", [33, 128], F32, isOutput=False)
    w1_d = nc.declare_dram_parameter("w1", [96, 128], BF16, isOutput=False)
    w2_d = nc.declare_dram_parameter("w2", [128, 4], BF16, isOutput=False)
    b2_d = nc.declare_dram_parameter("b2col", [128, 1], F32, isOutput=False)
    id_d = nc.declare_dram_parameter("ident", [6, 6], F32, isOutput=False)
    one_d = nc.declare_dram_parameter("one", [1, 1], F32, isOutput=False)
    out_d = nc.declare_dram_parameter("out", [12, XT_FREE], F32, isOutput=True)

    Gelu = mybir.ActivationFunctionType.Gelu

    with tile.TileContext(nc) as tc:
        with (
            tc.tile_pool(name="const", bufs=1) as cpool,
            tc.tile_pool(name="data", bufs=1) as dpool,
            tc.tile_pool(name="gp", bufs=3) as gpool,
            tc.tile_pool(name="dram", bufs=1, space="DRAM") as drpool,
        ):
            ind_sb = cpool.tile([128, 192], BF16)
            m2_sb = cpool.tile([128, 384], F32)
            wb_sb = cpool.tile([33, 128], F32)
            w1_sb = cpool.tile([96, 128], BF16)
            w2_sb = cpool.tile([128, 4], BF16)
            b2_sb = cpool.tile([128, 1], F32)
            id_sb = cpool.tile([6, 6], F32)
            acc1 = cpool.tile([33, 1], F32)
            bias_sb = cpool.tile([128, 1], F32)
            s_sb = cpool.tile([6, 256], F32)
            sT_sb = cpool.tile([128, 12], F32)

            # scan region rides the sync queue first; consts on scalar queue
            scan_sb = dpool.tile([128, SCAN_FREE], BF16)
            scan_dmas = []
            for q in range(4):
                scan_dmas.append(nc.sync.dma_start(
                    scan_sb[:, q * 2048:(q + 1) * 2048],
                    scan_d.ap()[:, q * 2048:(q + 1) * 2048],
                ))
            nc.scalar.dma_start(ind_sb[:], ind_d.ap())
            nc.scalar.dma_start(m2_sb[:], m2_d.ap())
            nc.scalar.dma_start(wb_sb[:], wb_d.ap())
            nc.scalar.dma_start(w1_sb[:], w1_d.ap())
            nc.scalar.dma_start(w2_sb[:], w2_d.ap())
            nc.scalar.dma_start(b2_sb[:], b2_d.ap())
            nc.scalar.dma_start(id_sb[:], id_d.ap())
            nc.scalar.dma_start(acc1[32:33, :], one_d.ap())
            xT_sb = dpool.tile([96, XT_FREE], BF16)
            for q in range(4):
                xi = nc.sync.dma_start(
                    xT_sb[:, q * 2432:(q + 1) * 2432],
                    xT_d.ap()[:, q * 2432:(q + 1) * 2432],
                )
                if q == 0:
                    # keep the scan transfer at full SDMA bandwidth: xT only
                    # starts once the scan region has landed
                    add_dep_helper(xi.ins, scan_dmas[-1].ins, sync=True,
                                   reason="serialize xT behind scan")
            stage_sb = dpool.tile([128, XT_FREE], F32)

            # ACT warm-up: force the gelu table load early
            warm_sb = cpool.tile([1, 8], F32)
            nc.gpsimd.memset(warm_sb[:], 0.0)
            nc.scalar.activation(warm_sb[:], warm_sb[:], Gelu)

            with tc.tile_pool(name="psA", bufs=1, space="PSUM") as psA:
                # stage 1: class sums over the full (replicated) scan region
                ps_s = psA.tile([6, 256], F32)
                for T in range(SCAN_TILES):
                    nc.tensor.matmul(
                        ps_s[:],
                        ind_sb[:, 6 * T:6 * T + 6],
                        scan_sb[:, 256 * T:256 * (T + 1)],
                        start=(T == 0),
                        stop=(T == SCAN_TILES - 1),
                    )
                nc.vector.tensor_copy(s_sb[:], ps_s[:])

                # transpose s (6,256) -> sT (128,12) via two identity matmuls
                ps_T = psA.tile([128, 12], F32)
                for jhi in range(2):
                    nc.tensor.matmul(
                        ps_T[:, 6 * jhi:6 * jhi + 6],
                        s_sb[:, 128 * jhi:128 * (jhi + 1)],
                        id_sb[:],
                        start=True,
                        stop=True,
                    )
                nc.vector.tensor_copy(sT_sb[:], ps_T[:])

                # acc = sum_k M2_k @ sT[:, k]
                ps_acc = psA.tile([32, 1], F32)
                for k in range(12):
                    nc.tensor.matmul(
                        ps_acc[:],
                        m2_sb[:, 32 * k:32 * (k + 1)],
                        sT_sb[:, k:k + 1],
                        start=(k == 0),
                        stop=(k == 11),
                    )
                nc.vector.tensor_copy(acc1[0:32, :], ps_acc[:])

                # bias1_eff = W1cat.T @ acc + (b1cat + gamma_tot @ W1cat)
                ps_b = psA.tile([128, 1], F32)
                nc.tensor.matmul(ps_b[:], wb_sb[:], acc1[:], start=True, stop=True)
                nc.vector.tensor_copy(bias_sb[:], ps_b[:])

            with (
                tc.tile_pool(name="psZ", bufs=2, space="PSUM") as zp,
                tc.tile_pool(name="psO", bufs=2, space="PSUM") as op,
            ):
                for t in range(NCH):
                    z = zp.tile([128, CH], F32)
                    for a in range(NSUB):
                        nc.tensor.matmul(
                            z[:, SUB * a:SUB * (a + 1)],
                            w1_sb[32 * a:32 * (a + 1), :],
                            xT_sb[32 * a:32 * (a + 1), SUB * t:SUB * (t + 1)],
                            start=True,
                            stop=True,
                            tile_position=(32 * a, 0),
                        )
                    g = gpool.tile([128, CH], BF16)
                    nc.scalar.activation(g[:], z[:], Gelu, bias=bias_sb[:])
                    o_ps = op.tile([128, SUB], F32)
                    for c in range(NSUB):
                        nc.tensor.matmul(
                            o_ps[32 * c:32 * c + 4, :],
                            w2_sb[:],
                            g[:, SUB * c:SUB * (c + 1)],
                            start=True,
                            stop=True,
                            tile_position=(0, 32 * c),
                        )
                    nc.vector.tensor_scalar_add(
                        stage_sb[:, SUB * t:SUB * (t + 1)], o_ps[:], b2_sb[:]
                    )
                    # batched output DMA on the idle gpsimd queue
                    if t in (4, 9, 14, 17, NCH - 1):
                        lo = {4: 0, 9: 2560, 14: 5120, 17: 7680, NCH - 1: 9216}[t]
                        hi = SUB * (t + 1)
                        for c in range(NSUB):
                            nc.gpsimd.dma_start(
                                out_d.ap()[4 * c:4 * c + 4, lo:hi],
                                stage_sb[32 * c:32 * c + 4, lo:hi],
                            )

    nc.compile()
    return nc


def _get_graph():
    global _GRAPH
    if _GRAPH is None:
        _GRAPH = _build_graph()
    return _GRAPH


# ---------------------------------------------------------------- kernel
def kernel(**inputs):
    import ml_dtypes
    from concourse import bass_utils

    data = np.asarray(inputs["data"], np.float32)
    conv_w = np.asarray(inputs["conv_w"], np.float32)
    conv_b = np.asarray(inputs["conv_b"], np.float32)
    dw = np.asarray(inputs["depth_weight"], np.float32)
    f_w1 = np.asarray(inputs["f_w1"], np.float32)
    f_b1 = np.asarray(inputs["f_b1"], np.float32)
    f_w2 = np.asarray(inputs["f_w2"], np.float32)
    f_b2 = np.asarray(inputs["f_b2"], np.float32)
    s_w1 = np.asarray(inputs["s_w1"], np.float32)
    s_b1 = np.asarray(inputs["s_b1"], np.float32)
    s_w2 = np.asarray(inputs["s_w2"], np.float32)
    s_b2 = np.asarray(inputs["s_b2"], np.float32)

    # --- weight-derived host constants (no data-sized work here) ---
    mats = _build_class_mats(conv_w, conv_b, dw)

    W1cat = np.concatenate([f_w1, s_w1], axis=1)          # (32, 128)
    b1cat = np.concatenate([f_b1, s_b1])                  # (128,)
    gamma_tot = np.zeros(D)
    for (p_lo, p_hi, M, gamma) in mats:
        gamma_tot += (p_hi - p_lo + 1) * gamma
    bconst = b1cat.astype(np.float64) + gamma_tot @ W1cat.astype(np.float64)
    WB = np.concatenate(
        [W1cat, bconst[None, :].astype(np.float32)], axis=0
    ).astype(np.float32)                                   # (33, 128)

    W2cat = np.zeros((128, 4), np.float32)
    W2cat[0:64, 0:3] = f_w2
    W2cat[64:128, 3:4] = s_w2
    b2cat = np.concatenate([f_b2, s_b2]).astype(np.float32)
    b2col = np.zeros((128, 1), np.float32)
    for c in range(NSUB):
        b2col[32 * c:32 * c + 4, 0] = b2cat

    # M2 (128, 384): col block k=6*jhi+d : M2[jlo, 32k+o] = M_d[o, 128*jhi+jlo]
    m2 = np.zeros((128, 384), np.float32)
    for dcls, (p_lo, p_hi, M, gamma) in enumerate(mats):
        Mf = M.astype(np.float32)
        for jhi in range(2):
            k = 6 * jhi + dcls
            m2[:, 32 * k:32 * (k + 1)] = Mf[:, 128 * jhi:128 * (jhi + 1)].T

    # --- shards ---
    data_flat = data.reshape(N_CELLS, D)
    w1_np = np.ascontiguousarray(
        np.tile(W1cat, (3, 1)).astype(ml_dtypes.bfloat16)
    )

    # replicated scan region (all 4096 parent nodes), bf16
    scan = np.ascontiguousarray(
        data_flat[0:N_GROUPS * 8].reshape(SCAN_TILES, 128, 256).transpose(1, 0, 2)
        .reshape(128, SCAN_FREE).astype(ml_dtypes.bfloat16)
    )
    ind = np.zeros((128, 192), np.float32)
    for dcls, (p_lo, p_hi, M, gamma) in enumerate(mats):
        for node in range(p_lo, p_hi + 1):
            T, p = divmod(node, 128)
            ind[p, 6 * T + dcls] = 1.0
    ind = ind.astype(ml_dtypes.bfloat16)

    in_maps = []
    for i in range(CORES):
        base = LEAF0 + CELLS_MAIN * i
        end = min(base + CELLS_CORE, N_CELLS)
        x_lin = np.zeros((CELLS_CORE, D), np.float32)
        x_lin[0:end - base] = data_flat[base:end]
        xT = np.ascontiguousarray(
            x_lin.reshape(NCH, NSUB, SUB, D).transpose(1, 3, 0, 2)
            .reshape(96, XT_FREE).astype(ml_dtypes.bfloat16)
        )
        in_maps.append({
            "xT": xT,
            "scanX": scan,
            "ind": ind,
            "m2": m2,
            "wb": WB,
            "w1": w1_np,
            "w2": W2cat.astype(ml_dtypes.bfloat16),
            "b2col": b2col,
            "ident": np.eye(6, dtype=np.float32),
            "one": np.ones((1, 1), np.float32),
        })

    nc = _get_graph()
    res = bass_utils.run_bass_kernel_spmd(nc, in_maps, core_ids=list(range(CORES)))

    out_flat = np.zeros((N_CELLS, 4), np.float32)
    for i in range(CORES):
        base = LEAF0 + CELLS_MAIN * i
        k = CELLS_MAIN if i < CORES - 1 else CELLS_MAIN + 1
        # planes (12, 9728): row 4c+o holds cells 1536t+512c+cc at free 512t+cc
        planes = res.results[i]["out"].reshape(NSUB, 4, NCH, SUB)  # (c,o,t,cc)
        cells = planes.transpose(2, 0, 3, 1).reshape(CELLS_CORE, 4)  # (t,c,cc),o
        out_flat[base:base + k] = cells[:k]
    return out_flat.reshape(N_NODES, 2, 2, 2, 4)


# revision 10
# speedup vs baseline: 1.0806x; 1.0427x over previous
"""Trainium2 Bass kernel for nn_AdExternal_N3Tree (gnn_message_passing).

Key insight: the reference's sequential 32768-step scan only affects the
output through `acc` (a 32-vector). Each parent's 8-child sibling group is an
independent serial chain that is LINEAR in that parent's original 8x32 block,
and group depth is constant within each of 6 contiguous parent-index classes.
So  acc = sum_d M_d @ s_d + gamma_tot,  where s_d is the sum of flattened
parent blocks over class d (a pure 4MB reduction) and M_d (32x256) / gamma
are tiny matrices computed on the host from conv_w/conv_b/depth_weight alone.

The leaf cells that feed the MLPs (flat cells 32767..262143) are never
written by the scan, so  out[leaf] = MLP(data_flat[leaf] + acc)  and cells
0..32766 are zero.

Device work per core (SPMD on 8 cores):
  - stage 1: partial class sums over a 512-node scan shard (f32 matmuls
    against a 0/1 class-indicator), AllGather + on-PE sum -> s (6x256)
  - tiny matmuls: s -> transpose -> acc -> folded layer-1 bias (128,)
  - MLP over a 29184-cell slice in bf16: x@W1cat (row-tiled) -> GELU+bias on
    ScalarE (the throughput bound, f32 PSUM in) -> @W2cat (col-tiled) ->
    +b2 evict on DVE
  - output written as 12 (c,o)-planes; host interleaves/assembles.
"""
import sys

for _p in ("/opt/trn_rl_repo", "/opt/trn_rl_repo/concourse"):
    if _p not in sys.path:
        sys.path.insert(0, _p)

import numpy as np

N_NODES = 32768
D = 32
N_GROUPS = 4096
N_CELLS = N_NODES * 8           # 262144
LEAF0 = N_NODES - 1             # 32767 first leaf cell
CORES = 8
CELLS_MAIN = 28672              # cells each core contributes (core 7: +1)
CELLS_CORE = 29184              # cells each core processes = 19 * 1536
CH = 1536                       # cells per chunk (3 row-tiled sub-chunks of 512)
NCH = 19
SUB = 512
NSUB = 3
SCAN_TILES = 32                 # replicated scan: 32 node-tiles of 128
SCAN_FREE = SCAN_TILES * 256    # 8192
XT_FREE = NCH * SUB             # 9728

# (p_lo, p_hi_inclusive, conv_depth, n_children, extra_j0_step)
CLASSES = [
    (0, 0, 1, 8, True),
    (1, 8, 2, 8, False),
    (9, 72, 3, 8, False),
    (73, 584, 4, 8, False),
    (585, 4094, 5, 8, False),
    (4095, 4095, 5, 7, False),
]


# ---------------------------------------------------------------- host math
def _chain(conv_w_d, conv_b_d, dw_d, n_children):
    W = conv_w_d.astype(np.float64)
    b = conv_b_d.astype(np.float64)
    Wk = [W[:, :, k] for k in range(8)]
    A, beta = {}, {}
    if n_children == 7:
        A7 = np.zeros((8, D, D))
        A7[7] = np.eye(D)
        A[7] = A7
        beta[7] = np.zeros(D)
        cs = range(6, -1, -1)
    else:
        cs = range(7, -1, -1)
    for c in cs:
        Ac = np.zeros((8, D, D))
        bc = b.copy()
        for k in range(0, c + 1):
            Ac[k] += Wk[k]
        for m in range(c + 1, 8):
            for k in range(8):
                Ac[k] += Wk[m] @ A[m][k]
            bc += Wk[m] @ beta[m]
        A[c] = Ac
        beta[c] = bc
    Msum = np.zeros((8, D, D))
    gamma = np.zeros(D)
    for c in (range(8) if n_children == 8 else range(7)):
        Msum += dw_d * A[c]
        gamma += dw_d * beta[c]
    return A, beta, Msum, gamma


def _build_class_mats(conv_w, conv_b, depth_weight):
    out = []
    for (p_lo, p_hi, dep, nch, extra) in CLASSES:
        A, beta, Msum, gamma = _chain(
            conv_w[dep], conv_b[dep], float(depth_weight[dep]), nch
        )
        if extra:
            W0 = conv_w[0].astype(np.float64)
            b0 = conv_b[0].astype(np.float64)
            W0k = [W0[:, :, k] for k in range(8)]
            Ae = np.zeros((8, D, D))
            be = b0.copy()
            for m in range(8):
                for k in range(8):
                    Ae[k] += W0k[m] @ A[m][k]
                be += W0k[m] @ beta[m]
            Msum = Msum + float(depth_weight[0]) * Ae
            gamma = gamma + float(depth_weight[0]) * be
        M = np.concatenate([Msum[k] for k in range(8)], axis=1)  # (D, 8D)
        out.append((p_lo, p_hi, M, gamma))
    return out


# ---------------------------------------------------------------- device graph
_GRAPH = None


def _build_graph():
    import concourse.bacc as bacc
    import concourse.mybir as mybir
    from concourse import tile
    from concourse.tile_rust import add_dep_helper

    F32 = mybir.dt.float32
    BF16 = mybir.dt.bfloat16
    nc = bacc.Bacc("TRN2", target_bir_lowering=False, debug=False, num_devices=CORES)

    xT_d = nc.declare_dram_parameter("xT", [96, XT_FREE], BF16, isOutput=False)
    scan_d = nc.declare_dram_parameter("scanX", [128, SCAN_FREE], BF16, isOutput=False)
    ind_d = nc.declare_dram_parameter("ind", [128, 192], BF16, isOutput=False)
    m2_d = nc.declare_dram_parameter("m2", [128, 384], F32, isOutput=False)
    wb_d = nc.declare_dram_parameter("wb", [33, 128], F32, isOutput=False)
    w1_d = nc.declare_dram_parameter("w1", [96, 128], BF16, isOutput=False)
    w2_d = nc.declare_dram_parameter("w2", [128, 4], BF16, isOutput=False)
    b2_d = nc.declare_dram_parameter("b2col", [128, 1], F32, isOutput=False)
    id_d = nc.declare_dram_parameter("ident", [6, 6], F32, isOutput=False)
    one_d = nc.declare_dram_parameter("one", [1, 1], F32, isOutput=False)
    out_d = nc.declare_dram_parameter("out", [12, XT_FREE], F32, isOutput=True)

    Gelu = mybir.ActivationFunctionType.Gelu

    with tile.TileContext(nc) as tc:
        with (
            tc.tile_pool(name="const", bufs=1) as cpool,
            tc.tile_pool(name="data", bufs=1) as dpool,
            tc.tile_pool(name="gp", bufs=3) as gpool,
            tc.tile_pool(name="dram", bufs=1, space="DRAM") as drpool,
        ):
            ind_sb = cpool.tile([128, 192], BF16)
            m2_sb = cpool.tile([128, 384], F32)
            wb_sb = cpool.tile([33, 128], F32)
            w1_sb = cpool.tile([96, 128], BF16)
            w2_sb = cpool.tile([128, 4], BF16)
            b2_sb = cpool.tile([128, 1], F32)
            id_sb = cpool.tile([6, 6], F32)
            acc1 = cpool.tile([33, 1], F32)
            bias_sb = cpool.tile([128, 1], F32)
            s_sb = cpool.tile([6, 256], F32)
            sT_sb = cpool.tile([128, 12], F32)

            # scan region rides the sync queue first; consts on scalar queue
            scan_sb = dpool.tile([128, SCAN_FREE], BF16)
            scan_dmas = []
            for q in range(4):
                scan_dmas.append(nc.sync.dma_start(
                    scan_sb[:, q * 2048:(q + 1) * 2048],
                    scan_d.ap()[:, q * 2048:(q + 1) * 2048],
                ))
            nc.scalar.dma_start(ind_sb[:], ind_d.ap())
            nc.scalar.dma_start(m2_sb[:], m2_d.ap())
            nc.scalar.dma_start(wb_sb[:], wb_d.ap())
            nc.scalar.dma_start(w1_sb[:], w1_d.ap())
            nc.scalar.dma_start(w2_sb[:], w2_d.ap())
            nc.scalar.dma_start(b2_sb[:], b2_d.ap())
            nc.scalar.dma_start(id_sb[:], id_d.ap())
            nc.scalar.dma_start(acc1[32:33, :], one_d.ap())
            xT_sb = dpool.tile([96, XT_FREE], BF16)
            for q in range(4):
                xi = nc.sync.dma_start(
                    xT_sb[:, q * 2432:(q + 1) * 2432],
                    xT_d.ap()[:, q * 2432:(q + 1) * 2432],
                )
                # keep the scan transfer at full SDMA bandwidth: xT only
                # starts once the scan region has landed
                add_dep_helper(xi.ins, scan_dmas[-1].ins, sync=True,
                               reason="serialize xT behind scan")
            stage_sb = dpool.tile([128, XT_FREE], F32)

            # ACT warm-up: force the gelu table load early
            warm_sb = cpool.tile([1, 8], F32)
            nc.gpsimd.memset(warm_sb[:], 0.0)
            nc.scalar.activation(warm_sb[:], warm_sb[:], Gelu)

            with tc.tile_pool(name="psA", bufs=1, space="PSUM") as psA:
                # stage 1: class sums over the full (replicated) scan region
                ps_s = psA.tile([6, 256], F32)
                for T in range(SCAN_TILES):
                    nc.tensor.matmul(
                        ps_s[:],
                        ind_sb[:, 6 * T:6 * T + 6],
                        scan_sb[:, 256 * T:256 * (T + 1)],
                        start=(T == 0),
                        stop=(T == SCAN_TILES - 1),
                    )
                nc.vector.tensor_copy(s_sb[:], ps_s[:])

                # transpose s (6,256) -> sT (128,12) via two identity matmuls
                ps_T = psA.tile([128, 12], F32)
                for jhi in range(2):
                    nc.tensor.matmul(
                        ps_T[:, 6 * jhi:6 * jhi + 6],
                        s_sb[:, 128 * jhi:128 * (jhi + 1)],
                        id_sb[:],
                        start=True,
                        stop=True,
                    )
                nc.vector.tensor_copy(sT_sb[:], ps_T[:])

                # acc = sum_k M2_k @ sT[:, k]
                ps_acc = psA.tile([32, 1], F32)
                for k in range(12):
                    nc.tensor.matmul(
                        ps_acc[:],
                        m2_sb[:, 32 * k:32 * (k + 1)],
                        sT_sb[:, k:k + 1],
                        start=(k == 0),
                        stop=(k == 11),
                    )
                nc.vector.tensor_copy(acc1[0:32, :], ps_acc[:])

                # bias1_eff = W1cat.T @ acc + (b1cat + gamma_tot @ W1cat)
                ps_b = psA.tile([128, 1], F32)
                nc.tensor.matmul(ps_b[:], wb_sb[:], acc1[:], start=True, stop=True)
                nc.vector.tensor_copy(bias_sb[:], ps_b[:])

            with (
                tc.tile_pool(name="psZ", bufs=2, space="PSUM") as zp,
                tc.tile_pool(name="psO", bufs=2, space="PSUM") as op,
            ):
                for t in range(NCH):
                    z = zp.tile([128, CH], F32)
                    for a in range(NSUB):
                        nc.tensor.matmul(
                            z[:, SUB * a:SUB * (a + 1)],
                            w1_sb[32 * a:32 * (a + 1), :],
                            xT_sb[32 * a:32 * (a + 1), SUB * t:SUB * (t + 1)],
                            start=True,
                            stop=True,
                            tile_position=(32 * a, 0),
                        )
                    g = gpool.tile([128, CH], BF16)
                    nc.scalar.activation(g[:], z[:], Gelu, bias=bias_sb[:])
                    o_ps = op.tile([128, SUB], F32)
                    for c in range(NSUB):
                        nc.tensor.matmul(
                            o_ps[32 * c:32 * c + 4, :],
                            w2_sb[:],
                            g[:, SUB * c:SUB * (c + 1)],
                            start=True,
                            stop=True,
                            tile_position=(0, 32 * c),
                        )
                    nc.vector.tensor_scalar_add(
                        stage_sb[:, SUB * t:SUB * (t + 1)], o_ps[:], b2_sb[:]
                    )
                    # batched output DMA on the idle gpsimd queue
                    if t in (4, 9, 14, 17, NCH - 1):
                        lo = {4: 0, 9: 2560, 14: 5120, 17: 7680, NCH - 1: 9216}[t]
                        hi = SUB * (t + 1)
                        for c in range(NSUB):
                            nc.gpsimd.dma_start(
                                out_d.ap()[4 * c:4 * c + 4, lo:hi],
                                stage_sb[32 * c:32 * c + 4, lo:hi],
                            )

    nc.compile()
    return nc


def _get_graph():
    global _GRAPH
    if _GRAPH is None:
        _GRAPH = _build_graph()
    return _GRAPH


# ---------------------------------------------------------------- kernel
def kernel(**inputs):
    import ml_dtypes
    from concourse import bass_utils

    data = np.asarray(inputs["data"], np.float32)
    conv_w = np.asarray(inputs["conv_w"], np.float32)
    conv_b = np.asarray(inputs["conv_b"], np.float32)
    dw = np.asarray(inputs["depth_weight"], np.float32)
    f_w1 = np.asarray(inputs["f_w1"], np.float32)
    f_b1 = np.asarray(inputs["f_b1"], np.float32)
    f_w2 = np.asarray(inputs["f_w2"], np.float32)
    f_b2 = np.asarray(inputs["f_b2"], np.float32)
    s_w1 = np.asarray(inputs["s_w1"], np.float32)
    s_b1 = np.asarray(inputs["s_b1"], np.float32)
    s_w2 = np.asarray(inputs["s_w2"], np.float32)
    s_b2 = np.asarray(inputs["s_b2"], np.float32)

    # --- weight-derived host constants (no data-sized work here) ---
    mats = _build_class_mats(conv_w, conv_b, dw)

    W1cat = np.concatenate([f_w1, s_w1], axis=1)          # (32, 128)
    b1cat = np.concatenate([f_b1, s_b1])                  # (128,)
    gamma_tot = np.zeros(D)
    for (p_lo, p_hi, M, gamma) in mats:
        gamma_tot += (p_hi - p_lo + 1) * gamma
    bconst = b1cat.astype(np.float64) + gamma_tot @ W1cat.astype(np.float64)
    WB = np.concatenate(
        [W1cat, bconst[None, :].astype(np.float32)], axis=0
    ).astype(np.float32)                                   # (33, 128)

    W2cat = np.zeros((128, 4), np.float32)
    W2cat[0:64, 0:3] = f_w2
    W2cat[64:128, 3:4] = s_w2
    b2cat = np.concatenate([f_b2, s_b2]).astype(np.float32)
    b2col = np.zeros((128, 1), np.float32)
    for c in range(NSUB):
        b2col[32 * c:32 * c + 4, 0] = b2cat

    # M2 (128, 384): col block k=6*jhi+d : M2[jlo, 32k+o] = M_d[o, 128*jhi+jlo]
    m2 = np.zeros((128, 384), np.float32)
    for dcls, (p_lo, p_hi, M, gamma) in enumerate(mats):
        Mf = M.astype(np.float32)
        for jhi in range(2):
            k = 6 * jhi + dcls
            m2[:, 32 * k:32 * (k + 1)] = Mf[:, 128 * jhi:128 * (jhi + 1)].T

    # --- shards ---
    data_flat = data.reshape(N_CELLS, D)
    w1_np = np.ascontiguousarray(
        np.tile(W1cat, (3, 1)).astype(ml_dtypes.bfloat16)
    )

    # replicated scan region (all 4096 parent nodes), bf16
    scan = np.ascontiguousarray(
        data_flat[0:N_GROUPS * 8].reshape(SCAN_TILES, 128, 256).transpose(1, 0, 2)
        .reshape(128, SCAN_FREE).astype(ml_dtypes.bfloat16)
    )
    ind = np.zeros((128, 192), np.float32)
    for dcls, (p_lo, p_hi, M, gamma) in enumerate(mats):
        for node in range(p_lo, p_hi + 1):
            T, p = divmod(node, 128)
            ind[p, 6 * T + dcls] = 1.0
    ind = ind.astype(ml_dtypes.bfloat16)

    in_maps = []
    for i in range(CORES):
        base = LEAF0 + CELLS_MAIN * i
        end = min(base + CELLS_CORE, N_CELLS)
        x_lin = np.zeros((CELLS_CORE, D), np.float32)
        x_lin[0:end - base] = data_flat[base:end]
        xT = np.ascontiguousarray(
            x_lin.reshape(NCH, NSUB, SUB, D).transpose(1, 3, 0, 2)
            .reshape(96, XT_FREE).astype(ml_dtypes.bfloat16)
        )
        in_maps.append({
            "xT": xT,
            "scanX": scan,
            "ind": ind,
            "m2": m2,
            "wb": WB,
            "w1": w1_np,
            "w2": W2cat.astype(ml_dtypes.bfloat16),
            "b2col": b2col,
            "ident": np.eye(6, dtype=np.float32),
            "one": np.ones((1, 1), np.float32),
        })

    nc = _get_graph()
    res = bass_utils.run_bass_kernel_spmd(nc, in_maps, core_ids=list(range(CORES)))

    out_flat = np.zeros((N_CELLS, 4), np.float32)
    for i in range(CORES):
        base = LEAF0 + CELLS_MAIN * i
        k = CELLS_MAIN if i < CORES - 1 else CELLS_MAIN + 1
        # planes (12, 9728): row 4c+o holds cells 1536t+512c+cc at free 512t+cc
        planes = res.results[i]["out"].reshape(NSUB, 4, NCH, SUB)  # (c,o,t,cc)
        cells = planes.transpose(2, 0, 3, 1).reshape(CELLS_CORE, 4)  # (t,c,cc),o
        out_flat[base:base + k] = cells[:k]
    return out_flat.reshape(N_NODES, 2, 2, 2, 4)


# revision 11
# speedup vs baseline: 1.1190x; 1.0356x over previous
"""Trainium2 Bass kernel for nn_AdExternal_N3Tree (gnn_message_passing).

Key insight: the reference's sequential 32768-step scan only affects the
output through `acc` (a 32-vector). Each parent's 8-child sibling group is an
independent serial chain that is LINEAR in that parent's original 8x32 block,
and group depth is constant within each of 6 contiguous parent-index classes.
So  acc = sum_d M_d @ s_d + gamma_tot,  where s_d is the sum of flattened
parent blocks over class d (a pure 4MB reduction) and M_d (32x256) / gamma
are tiny matrices computed on the host from conv_w/conv_b/depth_weight alone.

The leaf cells that feed the MLPs (flat cells 32767..262143) are never
written by the scan, so  out[leaf] = MLP(data_flat[leaf] + acc)  and cells
0..32766 are zero.

Device work per core (SPMD on 8 cores):
  - stage 1: partial class sums over a 512-node scan shard (f32 matmuls
    against a 0/1 class-indicator), AllGather + on-PE sum -> s (6x256)
  - tiny matmuls: s -> transpose -> acc -> folded layer-1 bias (128,)
  - MLP over a 29184-cell slice in bf16: x@W1cat (row-tiled) -> GELU+bias on
    ScalarE (the throughput bound, f32 PSUM in) -> @W2cat (col-tiled) ->
    +b2 evict on DVE
  - output written as 12 (c,o)-planes; host interleaves/assembles.
"""
import sys

for _p in ("/opt/trn_rl_repo", "/opt/trn_rl_repo/concourse"):
    if _p not in sys.path:
        sys.path.insert(0, _p)

import numpy as np

N_NODES = 32768
D = 32
N_GROUPS = 4096
N_CELLS = N_NODES * 8           # 262144
LEAF0 = N_NODES - 1             # 32767 first leaf cell
CORES = 8
CELLS_MAIN = 28672              # cells each core contributes (core 7: +1)
CELLS_CORE = 29184              # cells each core processes = 19 * 1536
CH = 1536                       # cells per chunk (3 row-tiled sub-chunks of 512)
NCH = 19
SUB = 512
NSUB = 3
SCAN_TILES = 32                 # replicated scan: 32 node-tiles of 128
SCAN_FREE = SCAN_TILES * 256    # 8192
XT_FREE = NCH * SUB             # 9728

# (p_lo, p_hi_inclusive, conv_depth, n_children, extra_j0_step)
CLASSES = [
    (0, 0, 1, 8, True),
    (1, 8, 2, 8, False),
    (9, 72, 3, 8, False),
    (73, 584, 4, 8, False),
    (585, 4094, 5, 8, False),
    (4095, 4095, 5, 7, False),
]


# ---------------------------------------------------------------- host math
def _chain(conv_w_d, conv_b_d, dw_d, n_children):
    W = conv_w_d.astype(np.float64)
    b = conv_b_d.astype(np.float64)
    Wk = [W[:, :, k] for k in range(8)]
    A, beta = {}, {}
    if n_children == 7:
        A7 = np.zeros((8, D, D))
        A7[7] = np.eye(D)
        A[7] = A7
        beta[7] = np.zeros(D)
        cs = range(6, -1, -1)
    else:
        cs = range(7, -1, -1)
    for c in cs:
        Ac = np.zeros((8, D, D))
        bc = b.copy()
        for k in range(0, c + 1):
            Ac[k] += Wk[k]
        for m in range(c + 1, 8):
            for k in range(8):
                Ac[k] += Wk[m] @ A[m][k]
            bc += Wk[m] @ beta[m]
        A[c] = Ac
        beta[c] = bc
    Msum = np.zeros((8, D, D))
    gamma = np.zeros(D)
    for c in (range(8) if n_children == 8 else range(7)):
        Msum += dw_d * A[c]
        gamma += dw_d * beta[c]
    return A, beta, Msum, gamma


def _build_class_mats(conv_w, conv_b, depth_weight):
    out = []
    for (p_lo, p_hi, dep, nch, extra) in CLASSES:
        A, beta, Msum, gamma = _chain(
            conv_w[dep], conv_b[dep], float(depth_weight[dep]), nch
        )
        if extra:
            W0 = conv_w[0].astype(np.float64)
            b0 = conv_b[0].astype(np.float64)
            W0k = [W0[:, :, k] for k in range(8)]
            Ae = np.zeros((8, D, D))
            be = b0.copy()
            for m in range(8):
                for k in range(8):
                    Ae[k] += W0k[m] @ A[m][k]
                be += W0k[m] @ beta[m]
            Msum = Msum + float(depth_weight[0]) * Ae
            gamma = gamma + float(depth_weight[0]) * be
        M = np.concatenate([Msum[k] for k in range(8)], axis=1)  # (D, 8D)
        out.append((p_lo, p_hi, M, gamma))
    return out


# ---------------------------------------------------------------- device graph
_GRAPH = None


def _build_graph():
    import concourse.bacc as bacc
    import concourse.mybir as mybir
    from concourse import tile
    from concourse.tile_rust import add_dep_helper

    F32 = mybir.dt.float32
    BF16 = mybir.dt.bfloat16
    nc = bacc.Bacc("TRN2", target_bir_lowering=False, debug=False, num_devices=CORES)

    xT_d = nc.declare_dram_parameter("xT", [96, XT_FREE], BF16, isOutput=False)
    scan_d = nc.declare_dram_parameter("scanX", [128, SCAN_FREE], BF16, isOutput=False)
    ind_d = nc.declare_dram_parameter("ind", [128, 192], BF16, isOutput=False)
    m2_d = nc.declare_dram_parameter("m2", [128, 384], F32, isOutput=False)
    wb_d = nc.declare_dram_parameter("wb", [33, 128], F32, isOutput=False)
    w1_d = nc.declare_dram_parameter("w1", [96, 128], BF16, isOutput=False)
    w2_d = nc.declare_dram_parameter("w2", [128, 4], BF16, isOutput=False)
    b2_d = nc.declare_dram_parameter("b2col", [128, 1], F32, isOutput=False)
    id_d = nc.declare_dram_parameter("ident", [6, 6], F32, isOutput=False)
    one_d = nc.declare_dram_parameter("one", [1, 1], F32, isOutput=False)
    out_d = nc.declare_dram_parameter("out", [12, XT_FREE], F32, isOutput=True)

    Gelu = mybir.ActivationFunctionType.Gelu

    with tile.TileContext(nc) as tc:
        with (
            tc.tile_pool(name="const", bufs=1) as cpool,
            tc.tile_pool(name="data", bufs=1) as dpool,
            tc.tile_pool(name="gp", bufs=3) as gpool,
            tc.tile_pool(name="dram", bufs=1, space="DRAM") as drpool,
        ):
            ind_sb = cpool.tile([128, 192], BF16)
            m2_sb = cpool.tile([128, 384], F32)
            wb_sb = cpool.tile([33, 128], F32)
            w1_sb = cpool.tile([96, 128], BF16)
            w2_sb = cpool.tile([128, 4], BF16)
            b2_sb = cpool.tile([128, 1], F32)
            id_sb = cpool.tile([6, 6], F32)
            acc1 = cpool.tile([33, 1], F32)
            bias_sb = cpool.tile([128, 1], F32)
            s_sb = cpool.tile([6, 256], F32)
            sT_sb = cpool.tile([128, 12], F32)

            # scan region rides the sync queue first; consts on scalar queue
            scan_sb = dpool.tile([128, SCAN_FREE], BF16)
            scan_dmas = []
            for q in range(2):
                scan_dmas.append(nc.gpsimd.dma_start(
                    scan_sb[:, q * 4096:(q + 1) * 4096],
                    scan_d.ap()[:, q * 4096:(q + 1) * 4096],
                ))
            nc.scalar.dma_start(ind_sb[:], ind_d.ap())
            nc.scalar.dma_start(m2_sb[:], m2_d.ap())
            nc.scalar.dma_start(wb_sb[:], wb_d.ap())
            nc.scalar.dma_start(w1_sb[:], w1_d.ap())
            nc.scalar.dma_start(w2_sb[:], w2_d.ap())
            nc.scalar.dma_start(b2_sb[:], b2_d.ap())
            nc.scalar.dma_start(id_sb[:], id_d.ap())
            nc.scalar.dma_start(acc1[32:33, :], one_d.ap())
            xT_sb = dpool.tile([96, XT_FREE], BF16)
            for q in range(4):
                xi = nc.sync.dma_start(
                    xT_sb[:, q * 2432:(q + 1) * 2432],
                    xT_d.ap()[:, q * 2432:(q + 1) * 2432],
                )
                # keep the scan transfer at full SDMA bandwidth: xT only
                # starts once the scan region has landed
                add_dep_helper(xi.ins, scan_dmas[-1].ins, sync=True,
                               reason="serialize xT behind scan")
            stage_sb = dpool.tile([128, XT_FREE], F32)

            # ACT warm-up: force the gelu table load early
            warm_sb = cpool.tile([1, 8], F32)
            nc.gpsimd.memset(warm_sb[:], 0.0)
            nc.scalar.activation(warm_sb[:], warm_sb[:], Gelu)

            with tc.tile_pool(name="psA", bufs=1, space="PSUM") as psA:
                # stage 1: class sums over the full (replicated) scan region
                ps_s = psA.tile([6, 256], F32)
                for T in range(SCAN_TILES):
                    nc.tensor.matmul(
                        ps_s[:],
                        ind_sb[:, 6 * T:6 * T + 6],
                        scan_sb[:, 256 * T:256 * (T + 1)],
                        start=(T == 0),
                        stop=(T == SCAN_TILES - 1),
                    )
                nc.vector.tensor_copy(s_sb[:], ps_s[:])

                # transpose s (6,256) -> sT (128,12) via two identity matmuls
                ps_T = psA.tile([128, 12], F32)
                for jhi in range(2):
                    nc.tensor.matmul(
                        ps_T[:, 6 * jhi:6 * jhi + 6],
                        s_sb[:, 128 * jhi:128 * (jhi + 1)],
                        id_sb[:],
                        start=True,
                        stop=True,
                    )
                nc.vector.tensor_copy(sT_sb[:], ps_T[:])

                # acc = sum_k M2_k @ sT[:, k]
                ps_acc = psA.tile([32, 1], F32)
                for k in range(12):
                    nc.tensor.matmul(
                        ps_acc[:],
                        m2_sb[:, 32 * k:32 * (k + 1)],
                        sT_sb[:, k:k + 1],
                        start=(k == 0),
                        stop=(k == 11),
                    )
                nc.vector.tensor_copy(acc1[0:32, :], ps_acc[:])

                # bias1_eff = W1cat.T @ acc + (b1cat + gamma_tot @ W1cat)
                ps_b = psA.tile([128, 1], F32)
                nc.tensor.matmul(ps_b[:], wb_sb[:], acc1[:], start=True, stop=True)
                nc.vector.tensor_copy(bias_sb[:], ps_b[:])

            with (
                tc.tile_pool(name="psZ", bufs=2, space="PSUM") as zp,
                tc.tile_pool(name="psO", bufs=2, space="PSUM") as op,
            ):
                for t in range(NCH):
                    z = zp.tile([128, CH], F32)
                    for a in range(NSUB):
                        nc.tensor.matmul(
                            z[:, SUB * a:SUB * (a + 1)],
                            w1_sb[32 * a:32 * (a + 1), :],
                            xT_sb[32 * a:32 * (a + 1), SUB * t:SUB * (t + 1)],
                            start=True,
                            stop=True,
                            tile_position=(32 * a, 0),
                        )
                    g = gpool.tile([128, CH], BF16)
                    nc.scalar.activation(g[:], z[:], Gelu, bias=bias_sb[:])
                    o_ps = op.tile([128, SUB], F32)
                    for c in range(NSUB):
                        nc.tensor.matmul(
                            o_ps[32 * c:32 * c + 4, :],
                            w2_sb[:],
                            g[:, SUB * c:SUB * (c + 1)],
                            start=True,
                            stop=True,
                            tile_position=(0, 32 * c),
                        )
                    nc.vector.tensor_scalar_add(
                        stage_sb[:, SUB * t:SUB * (t + 1)], o_ps[:], b2_sb[:]
                    )
                    # batched output DMA on the idle gpsimd queue
                    if t in (4, 9, 14, 17, NCH - 1):
                        lo = {4: 0, 9: 2560, 14: 5120, 17: 7680, NCH - 1: 9216}[t]
                        hi = SUB * (t + 1)
                        for c in range(NSUB):
                            nc.gpsimd.dma_start(
                                out_d.ap()[4 * c:4 * c + 4, lo:hi],
                                stage_sb[32 * c:32 * c + 4, lo:hi],
                            )

    nc.compile()
    return nc


def _get_graph():
    global _GRAPH
    if _GRAPH is None:
        _GRAPH = _build_graph()
    return _GRAPH


# ---------------------------------------------------------------- kernel
def kernel(**inputs):
    import ml_dtypes
    from concourse import bass_utils

    data = np.asarray(inputs["data"], np.float32)
    conv_w = np.asarray(inputs["conv_w"], np.float32)
    conv_b = np.asarray(inputs["conv_b"], np.float32)
    dw = np.asarray(inputs["depth_weight"], np.float32)
    f_w1 = np.asarray(inputs["f_w1"], np.float32)
    f_b1 = np.asarray(inputs["f_b1"], np.float32)
    f_w2 = np.asarray(inputs["f_w2"], np.float32)
    f_b2 = np.asarray(inputs["f_b2"], np.float32)
    s_w1 = np.asarray(inputs["s_w1"], np.float32)
    s_b1 = np.asarray(inputs["s_b1"], np.float32)
    s_w2 = np.asarray(inputs["s_w2"], np.float32)
    s_b2 = np.asarray(inputs["s_b2"], np.float32)

    # --- weight-derived host constants (no data-sized work here) ---
    mats = _build_class_mats(conv_w, conv_b, dw)

    W1cat = np.concatenate([f_w1, s_w1], axis=1)          # (32, 128)
    b1cat = np.concatenate([f_b1, s_b1])                  # (128,)
    gamma_tot = np.zeros(D)
    for (p_lo, p_hi, M, gamma) in mats:
        gamma_tot += (p_hi - p_lo + 1) * gamma
    bconst = b1cat.astype(np.float64) + gamma_tot @ W1cat.astype(np.float64)
    WB = np.concatenate(
        [W1cat, bconst[None, :].astype(np.float32)], axis=0
    ).astype(np.float32)                                   # (33, 128)

    W2cat = np.zeros((128, 4), np.float32)
    W2cat[0:64, 0:3] = f_w2
    W2cat[64:128, 3:4] = s_w2
    b2cat = np.concatenate([f_b2, s_b2]).astype(np.float32)
    b2col = np.zeros((128, 1), np.float32)
    for c in range(NSUB):
        b2col[32 * c:32 * c + 4, 0] = b2cat

    # M2 (128, 384): col block k=6*jhi+d : M2[jlo, 32k+o] = M_d[o, 128*jhi+jlo]
    m2 = np.zeros((128, 384), np.float32)
    for dcls, (p_lo, p_hi, M, gamma) in enumerate(mats):
        Mf = M.astype(np.float32)
        for jhi in range(2):
            k = 6 * jhi + dcls
            m2[:, 32 * k:32 * (k + 1)] = Mf[:, 128 * jhi:128 * (jhi + 1)].T

    # --- shards ---
    data_flat = data.reshape(N_CELLS, D)
    w1_np = np.ascontiguousarray(
        np.tile(W1cat, (3, 1)).astype(ml_dtypes.bfloat16)
    )

    # replicated scan region (all 4096 parent nodes), bf16
    scan = np.ascontiguousarray(
        data_flat[0:N_GROUPS * 8].reshape(SCAN_TILES, 128, 256).transpose(1, 0, 2)
        .reshape(128, SCAN_FREE).astype(ml_dtypes.bfloat16)
    )
    ind = np.zeros((128, 192), np.float32)
    for dcls, (p_lo, p_hi, M, gamma) in enumerate(mats):
        for node in range(p_lo, p_hi + 1):
            T, p = divmod(node, 128)
            ind[p, 6 * T + dcls] = 1.0
    ind = ind.astype(ml_dtypes.bfloat16)

    in_maps = []
    for i in range(CORES):
        base = LEAF0 + CELLS_MAIN * i
        end = min(base + CELLS_CORE, N_CELLS)
        x_lin = np.zeros((CELLS_CORE, D), np.float32)
        x_lin[0:end - base] = data_flat[base:end]
        xT = np.ascontiguousarray(
            x_lin.reshape(NCH, NSUB, SUB, D).transpose(1, 3, 0, 2)
            .reshape(96, XT_FREE).astype(ml_dtypes.bfloat16)
        )
        in_maps.append({
            "xT": xT,
            "scanX": scan,
            "ind": ind,
            "m2": m2,
            "wb": WB,
            "w1": w1_np,
            "w2": W2cat.astype(ml_dtypes.bfloat16),
            "b2col": b2col,
            "ident": np.eye(6, dtype=np.float32),
            "one": np.ones((1, 1), np.float32),
        })

    nc = _get_graph()
    res = bass_utils.run_bass_kernel_spmd(nc, in_maps, core_ids=list(range(CORES)))

    out_flat = np.zeros((N_CELLS, 4), np.float32)
    for i in range(CORES):
        base = LEAF0 + CELLS_MAIN * i
        k = CELLS_MAIN if i < CORES - 1 else CELLS_MAIN + 1
        # planes (12, 9728): row 4c+o holds cells 1536t+512c+cc at free 512t+cc
        planes = res.results[i]["out"].reshape(NSUB, 4, NCH, SUB)  # (c,o,t,cc)
        cells = planes.transpose(2, 0, 3, 1).reshape(CELLS_CORE, 4)  # (t,c,cc),o
        out_flat[base:base + k] = cells[:k]
    return out_flat.reshape(N_NODES, 2, 2, 2, 4)
